# revision 1
# baseline (speedup 1.0000x reference)
"""Trainium2 Bass kernel for nn_EnhancedClassificationHead_53944789237866.

kernel(**inputs) takes the full unsharded inputs (as from setup_inputs())
and returns the full (16, 3) logits.  The heavy stage (2 top-20-masked
attention layers over 48 sequences of 512 tokens) runs on 8 NeuronCores,
batch-sharded 6 sequences per core; the tiny cross-attention + classifier
tail (48 tokens total) runs on host.
"""

import sys, os, json
for _p in ("/opt/trn_rl_repo",):
    if _p not in sys.path:
        sys.path.insert(0, _p)
import numpy as np
import ml_dtypes
from contextlib import ExitStack

# ---------------------------------------------------------------- compat ----


def fix_bir_multiwait(bir_bytes):
    m = json.loads(bir_bytes)
    changed = False
    for f in m.get("functions", []):
        for bb in f.get("basicblocks", f.get("blocks", [])):
            insts = bb["instructions"]
            out = []
            for inst in insts:
                si = inst.get("sync_info")
                waits = (si or {}).get("on_wait") or []
                if len(waits) > 1:
                    changed = True
                    for k, w in enumerate(waits[:-1]):
                        out.append({
                            "debug": inst.get("debug"),
                            "engine": inst["engine"],
                            "ins": [], "outs": [],
                            "name": f"{inst['name']}-ws{k}",
                            "opcode": "Drain",
                            "sync_info": {"on_update": [], "on_wait": [w]},
                        })
                    si["on_wait"] = [waits[-1]]
                out.append(inst)
            bb["instructions"] = out
    if not changed:
        return bir_bytes
    return json.dumps(m).encode()


_installed = False


def install():
    global _installed
    if _installed:
        return
    _installed = True
    import concourse.bass_utils as bu
    import concourse.bass2jax as b2j

    orig = bu.compile_bir_kernel

    def wrapped(bir_json, tmpdir, neff_name="file.neff"):
        return orig(fix_bir_multiwait(bir_json), tmpdir, neff_name)

    bu.compile_bir_kernel = wrapped
    b2j.compile_bir_kernel = wrapped


install()

import concourse.bass as bass
import concourse.mybir as mybir
import concourse.tile as tile
from concourse.bass_utils import run_bass_kernel_spmd

# --------------------------------------------------------------- builder ----


F32 = mybir.dt.float32
BF16 = mybir.dt.bfloat16
AX = mybir.AxisListType
ALU = mybir.AluOpType
ACTF = mybir.ActivationFunctionType

C = 256
N = 512
H = 8
HD = 32
TOPM = 20
NCT = 2   # c tiles (256/128)
NQT = 4   # q chunks (512/128)
P = 128


def build(nseq=6, nlayer=2, reps=1):
    nc = bass.Bass()

    feat = nc.dram_tensor("feat", [nseq, C, N], F32, kind="ExternalInput")
    pos = nc.dram_tensor("pos", [C, N], F32, kind="ExternalInput")
    wq = nc.dram_tensor("wq", [nlayer, C, C], BF16, kind="ExternalInput")
    wk = nc.dram_tensor("wk", [nlayer, C, C], BF16, kind="ExternalInput")
    wv = nc.dram_tensor("wv", [nlayer, C, C], BF16, kind="ExternalInput")
    wp = nc.dram_tensor("wp", [nlayer, C, C], BF16, kind="ExternalInput")
    bq = nc.dram_tensor("bq", [nlayer, C, 1], F32, kind="ExternalInput")
    bk = nc.dram_tensor("bk", [nlayer, C, 1], F32, kind="ExternalInput")
    bp2 = nc.dram_tensor("bp2", [nlayer, C, 1], F32, kind="ExternalInput")
    lng = nc.dram_tensor("lng", [nlayer, C, 1], F32, kind="ExternalInput")
    lnb = nc.dram_tensor("lnb", [nlayer, C, 1], F32, kind="ExternalInput")
    ident = nc.dram_tensor("ident", [128, 128], BF16, kind="ExternalInput")
    identf = nc.dram_tensor("identf", [128, 128], F32, kind="ExternalInput")
    pooled = nc.dram_tensor("pooled", [C, nseq], F32, kind="ExternalOutput")

    with ExitStack() as ctx:
        tc = ctx.enter_context(tile.TileContext(nc))
        _body(ctx, tc, nseq, nlayer, reps, feat, pos, wq, wk, wv, wp,
              bq, bk, bp2, lng, lnb, ident, identf, pooled)
    return nc


def _body(ctx, tc, nseq, nlayer, reps, feat, pos, wq, wk, wv, wp,
          bq, bk, bp2, lng, lnb, ident, identf, pooled):
    nc = tc.nc

    consts = ctx.enter_context(tc.tile_pool(name="consts", bufs=1))
    xpool = ctx.enter_context(tc.tile_pool(name="xpool", bufs=1))
    qkv = ctx.enter_context(tc.tile_pool(name="qkv", bufs=1))
    sbp = ctx.enter_context(tc.tile_pool(name="sbp", bufs=3))
    epool = ctx.enter_context(tc.tile_pool(name="epool", bufs=6))
    small = ctx.enter_context(tc.tile_pool(name="small", bufs=4))
    rowp = ctx.enter_context(tc.tile_pool(name="rowp", bufs=2))
    psA = ctx.enter_context(tc.tile_pool(name="psA", bufs=3, space="PSUM"))
    psB = ctx.enter_context(tc.tile_pool(name="psB", bufs=1, space="PSUM"))
    pools = dict(consts=consts, xpool=xpool, qkv=qkv, sbp=sbp, epool=epool,
                 small=small, rowp=rowp, psA=psA, psB=psB)

    # ---- constants ----
    id_t = consts.tile([128, 128], BF16)
    nc.sync.dma_start(out=id_t, in_=ident[:, :])
    id_f = consts.tile([128, 128], F32)
    nc.sync.dma_start(out=id_f, in_=identf[:, :])
    pos_t = [consts.tile([P, N], F32, tag=f"pos{_}", name=f"pos{_}") for _ in range(NCT)]
    for c in range(NCT):
        nc.sync.dma_start(out=pos_t[c], in_=pos[c * P:(c + 1) * P, :])
    ones_bf = consts.tile([128, 128], BF16)
    nc.vector.memset(ones_bf, 1.0)
    zero_c = consts.tile([P, 1], F32)
    nc.vector.memset(zero_c, 0.0)
    eps_c = consts.tile([P, 1], F32)
    nc.vector.memset(eps_c, 1e-5)

    W = {}
    for name, src in (("wq", wq), ("wk", wk), ("wv", wv), ("wp", wp)):
        W[name] = {}
        for l in range(nlayer):
            W[name][l] = [consts.tile([P, C], BF16, tag=f"{name}_{l}_{_}", name=f"{name}_{l}_{_}") for _ in range(NCT)]
            for c in range(NCT):
                nc.sync.dma_start(out=W[name][l][c],
                                  in_=src[l, c * P:(c + 1) * P, :])
    B = {}
    for name, src in (("bq", bq), ("bk", bk), ("bp2", bp2),
                      ("lng", lng), ("lnb", lnb)):
        B[name] = {}
        for l in range(nlayer):
            B[name][l] = [consts.tile([P, 1], F32, tag=f"{name}_{l}_{_}", name=f"{name}_{l}_{_}") for _ in range(NCT)]
            for c in range(NCT):
                nc.sync.dma_start(out=B[name][l][c],
                                  in_=src[l, c * P:(c + 1) * P, :])

    x_fm = [xpool.tile([P, N], BF16, tag=f"x_fm{_}", name=f"x_fm{_}") for _ in range(NCT)]
    pooled_sb = xpool.tile([P, NCT, nseq], F32, tag="pooled")

    for _rep in range(reps):
        for s in range(nseq):
            for c in range(NCT):
                ft = sbp.tile([P, N], F32, tag="featld")
                nc.sync.dma_start(out=ft, in_=feat[s, c * P:(c + 1) * P, :])
                nc.vector.tensor_add(x_fm[c], ft, pos_t[c])
            for l in range(nlayer):
                _layer(tc, l, x_fm, W, B, id_t, id_f, ones_bf, zero_c, eps_c, pools)
            for c in range(NCT):
                nc.vector.tensor_reduce(
                    out=pooled_sb[:, c, s:s + 1], in_=x_fm[c],
                    axis=AX.X, op=ALU.add)

    psc = small.tile([P, NCT, nseq], F32, tag="psc")
    nc.vector.tensor_scalar_mul(psc, pooled_sb, 1.0 / N)
    for c in range(NCT):
        nc.sync.dma_start(out=pooled[c * P:(c + 1) * P, :], in_=psc[:, c, :])


def _layer(tc, l, x_fm, W, B, id_t, id_f, ones_bf, zero_c, eps_c, pools):
    import os
    STAGE = int(os.environ.get("KB_STAGE", "9"))
    nc = tc.nc
    SCL = float(HD) ** -0.5
    qkv, sbp, epool, small, rowp = (pools[k] for k in
                                    ("qkv", "sbp", "epool", "small", "rowp"))
    psA, psB = (pools[k] for k in ("psA", "psB"))
    wq_t, wk_t, wv_t, wp_t = W["wq"][l], W["wk"][l], W["wv"][l], W["wp"][l]
    bq_t, bk_t, bp2_t = B["bq"][l], B["bk"][l], B["bp2"][l]
    lng_t, lnb_t = B["lng"][l], B["lnb"][l]

    # ---- QKV ----
    q_fm = [qkv.tile([P, N], BF16, tag=f"q_fm{_}", name=f"q_fm{_}") for _ in range(NCT)]
    k_fm = [qkv.tile([P, N], BF16, tag=f"k_fm{_}", name=f"k_fm{_}") for _ in range(NCT)]
    v_tm = [qkv.tile([P, C], BF16, tag=f"v_tm{_}", name=f"v_tm{_}") for _ in range(NQT)]
    for o in range(NCT):
        ps = psA.tile([P, N], F32, tag="big")
        for c in range(NCT):
            nc.tensor.matmul(ps, lhsT=wq_t[c][:, o * P:(o + 1) * P],
                             rhs=x_fm[c], start=(c == 0), stop=(c == NCT - 1))
        # q = (Wq x + bq) * 1/sqrt(HD)
        nc.vector.tensor_scalar(q_fm[o], ps, bq_t[o], SCL,
                                op0=ALU.add, op1=ALU.mult)
    for o in range(NCT):
        ps = psA.tile([P, N], F32, tag="big")
        for c in range(NCT):
            nc.tensor.matmul(ps, lhsT=wk_t[c][:, o * P:(o + 1) * P],
                             rhs=x_fm[c], start=(c == 0), stop=(c == NCT - 1))
        nc.vector.tensor_scalar(k_fm[o], ps, bk_t[o], None, op0=ALU.add)
    for t in range(NQT):
        ps = psA.tile([P, N], F32, tag="big")
        for c in range(NCT):
            nc.tensor.matmul(ps[:, 0:C], lhsT=x_fm[c][:, t * P:(t + 1) * P],
                             rhs=wv_t[c], start=(c == 0), stop=(c == NCT - 1))
        nc.vector.tensor_copy(v_tm[t], ps[:, 0:C])

    tneg = [small.tile([P, 2, P], F32, tag=f"tneg{_}", name=f"tneg{_}")
            for _ in range(NQT)]
    zbuf = [small.tile([P, 2, P], F32, tag=f"zbuf{_}", name=f"zbuf{_}")
            for _ in range(NQT)]
    for qc in range(NQT):
        nc.vector.memset(tneg[qc], 0.0)
        nc.vector.memset(zbuf[qc], 1.0)

    # ---- phase 1: S_tm -> top-20 threshold + Z per (h, qc) ----
    for qc in range(NQT):
        zcq = small.tile([P, H, 24], F32, tag=f"zcq{qc}", name=f"zcq{qc}",
                         bufs=1)
        for h in range(H):
            ht, hr = divmod(h, 4)
            qh = q_fm[ht][hr * 32:(hr + 1) * 32, :]
            kh = k_fm[ht][hr * 32:(hr + 1) * 32, :]
            ps = psA.tile([P, N], F32, tag="big")
            nc.tensor.matmul(ps, lhsT=qh[:, qc * P:(qc + 1) * P], rhs=kh,
                             start=True, stop=True,
                             tile_position=(hr * 32, 0))
            if STAGE < 3:
                continue
            s2 = sbp.tile([P, N], F32, tag="s2")
            nc.vector.max(out=zcq[:, h, 0:8], in_=ps)
            nc.vector.match_replace(out=s2, in_to_replace=zcq[:, h, 0:8],
                                    in_values=ps, imm_value=-1e30)
            nc.vector.max(out=zcq[:, h, 8:16], in_=s2)
            nc.vector.match_replace(out=s2, in_to_replace=zcq[:, h, 8:16],
                                    in_values=s2, imm_value=-1e30)
            nc.vector.max(out=zcq[:, h, 16:24], in_=s2)
        if STAGE < 3:
            continue
        # batched per-qc: t = zcq[:, :, 19]; tlo = |t|*2^-9; tneg = tlo - t
        tv = zcq[:, :, 19:20]
        tb = small.tile([P, H], F32, tag="tb", name=f"tb{qc}")
        nc.vector.tensor_scalar(tb, tv, -1.0, None, op0=ALU.mult)
        ta = small.tile([P, H], F32, tag="tab", name=f"tab{qc}")
        nc.vector.tensor_max(ta, tv, tb)
        nc.vector.tensor_scalar(tneg[qc][:, :, 1:98:32], ta, 2.0 ** -9, None,
                                op0=ALU.mult)
        nc.vector.scalar_tensor_tensor(
            out=tneg[qc][:, :, 0:97:32], in0=ta, scalar=2.0 ** -9,
            in1=tv, op0=ALU.mult, op1=ALU.subtract)
        # Z per head: sum over top-20 of exp(cand - t')
        zd = small.tile([P, 2, 4, TOPM], F32, tag="zd", name=f"zd{qc}")
        tnv = tneg[qc][:, :, 0:97:32]
        tnb = bass.AP(tensor=tnv.tensor, offset=tnv.offset,
                      ap=list(tnv.ap) + [[0, TOPM]])
        zin = zcq[:, :, 0:TOPM].rearrange("p (a b) t -> p a b t", a=2)
        nc.vector.tensor_add(zd, zin, tnb)
        zel = small.tile([P, 2, 4, TOPM], F32, tag="zel", name=f"zel{qc}")
        nc.scalar.activation(zel, zd, ACTF.Exp, bias=zero_c)
        nc.vector.tensor_reduce(out=zbuf[qc][:, :, 0:97:32], in_=zel,
                                axis=AX.X, op=ALU.add)

    # ---- assemble -t' / tlo / 1/Z as [128, 512] bf16, heads 32-aligned ----
    if STAGE < 4:
        return
    for qc in range(NQT):
        nc.vector.reciprocal(zbuf[qc], zbuf[qc])
    rows_sb = {}
    for tname, srcs in (("trow", tneg), ("zrow", zbuf)):
        rows_sb[tname] = []
        for g in range(2):
            prow = psA.tile([P, N], F32, tag="big", name=f"p{tname}{g}")
            for qc in range(NQT):
                nc.tensor.transpose(prow[:, qc * P:(qc + 1) * P],
                                    srcs[qc][:, g, :], id_f)
            srow = rowp.tile([P, N], BF16, tag=f"{tname}{g}",
                             name=f"{tname}{g}")
            nc.vector.tensor_copy(srow, prow)
            rows_sb[tname].append(srow)
    trow, zrow = (rows_sb[k] for k in ("trow", "zrow"))

    zrep_ps = [psA.tile([P, N], F32, tag="big", name=f"zrepps{_}") for _ in range(NCT)]
    for h in range(H):
        ht, hr = divmod(h, 4)
        b = hr * 32
        nc.tensor.matmul(zrep_ps[ht][b:b + 32, :],
                         lhsT=ones_bf[b:b + 1, 0:32],
                         rhs=zrow[ht][b:b + 1, :],
                         start=True, stop=True, tile_position=(b, b))
    zrep = [epool.tile([P, N], BF16, tag=f"zrep_sb{_}", name=f"zrep_sb{_}", bufs=1) for _ in range(NCT)]
    for c in range(NCT):
        nc.vector.tensor_copy(zrep[c], zrep_ps[c])

    # ---- phase 2: S~^T -> exp -> mask -> PV ----
    if STAGE < 5:
        return
    pv_ps = [psB.tile([P, N], F32, tag=f"pv{i}", name=f"pvps{i}") for i in range(NCT)]
    for h in range(H):
        ht, hr = divmod(h, 4)
        qh = q_fm[ht][hr * 32:(hr + 1) * 32, :]
        kh = k_fm[ht][hr * 32:(hr + 1) * 32, :]
        for kc in range(NQT):
            ps = psA.tile([P, N], F32, tag="big")
            nc.tensor.matmul(ps, lhsT=kh[:, kc * P:(kc + 1) * P], rhs=qh,
                             start=True, stop=False,
                             tile_position=(hr * 32, 0))
            b = hr * 32
            nc.tensor.matmul(ps, lhsT=ones_bf[b:b + 2, :],
                             rhs=trow[ht][b:b + 2, :],
                             start=False, stop=True, tile_position=(b, 0))
            if STAGE < 6:
                continue
            e2 = epool.tile([P, N], BF16, tag="e2")
            nc.scalar.activation(e2, ps, ACTF.Exp, bias=zero_c)
            em = epool.tile([P, N], BF16, tag="em")
            nc.vector.scalar_tensor_tensor(
                out=em, in0=ps, scalar=0.0, in1=e2,
                op0=ALU.is_ge, op1=ALU.mult)
            nc.tensor.matmul(pv_ps[ht][hr * 32:(hr + 1) * 32, :],
                             lhsT=v_tm[kc][:, h * 32:(h + 1) * 32], rhs=em,
                             start=(kc == 0), stop=(kc == NQT - 1),
                             tile_position=(0, hr * 32))

    # ---- normalize + proj + residual ----
    if STAGE < 7:
        return
    attn = [epool.tile([P, N], BF16, tag=f"attn{i}", name=f"attn{i}", bufs=1) for i in range(NCT)]
    for c in range(NCT):
        nc.vector.tensor_mul(attn[c], pv_ps[c], zrep[c])
    y_ps = [psA.tile([P, N], F32, tag="big", name=f"zrepps{_}") for _ in range(NCT)]
    for o in range(NCT):
        for c in range(NCT):
            nc.tensor.matmul(y_ps[o], lhsT=wp_t[c][:, o * P:(o + 1) * P],
                             rhs=attn[c], start=(c == 0), stop=(c == NCT - 1))
    r_fm = [epool.tile([P, N], BF16, tag=f"r_fm{i}", name=f"r_fm{i}", bufs=1) for i in range(NCT)]
    for c in range(NCT):
        nc.vector.scalar_tensor_tensor(
            out=r_fm[c], in0=y_ps[c], scalar=bp2_t[c], in1=x_fm[c],
            op0=ALU.add, op1=ALU.add)

    # ---- LN (token-major) + write back x_fm with gamma/beta ----
    if STAGE < 8:
        return
    xh_tms = []
    for qc in range(NQT):
        rtm = psA.tile([P, N], BF16, tag="bigb", bufs=1)
        for c in range(NCT):
            nc.tensor.transpose(rtm[:, c * P:(c + 1) * P],
                                r_fm[c][:, qc * P:(qc + 1) * P], id_t)
        stats = small.tile([P, 6], F32, tag="stats")
        mv = small.tile([P, 2], F32, tag="mv")
        nc.vector.bn_stats(out=stats, in_=rtm[:, 0:C])
        nc.vector.bn_aggr(out=mv, in_=stats)
        lnv = small.tile([P, 1], F32, tag="lnv")
        nc.scalar.activation(lnv, mv[:, 1:2], ACTF.Ln, bias=eps_c, scale=1.0)
        rstd = small.tile([P, 1], F32, tag="rstd")
        nc.scalar.activation(rstd, lnv, ACTF.Exp, bias=zero_c, scale=-0.5)
        xh_tm = sbp.tile([P, C], BF16, tag=f"xh_tm{qc}", bufs=1,
                         name=f"xh_tm{qc}")
        nc.vector.tensor_scalar(xh_tm, rtm[:, 0:C], mv[:, 0:1], rstd,
                                op0=ALU.subtract, op1=ALU.mult)
        xh_tms.append(xh_tm)
    for c in range(NCT):
        xf_ps = psA.tile([P, N], BF16, tag="xf", bufs=2, name=f"xfps{c}")
        for qc in range(NQT):
            nc.tensor.transpose(xf_ps[:, qc * P:(qc + 1) * P],
                                xh_tms[qc][:, c * P:(c + 1) * P], id_t)
        nc.vector.tensor_scalar(x_fm[c], xf_ps, lng_t[c], lnb_t[c],
                                op0=ALU.mult, op1=ALU.add)


# ----------------------------------------------------------- host helpers ----

BF = ml_dtypes.bfloat16


def bf16(x):
    return np.asarray(x, dtype=np.float32).astype(BF).astype(np.float32)


def prep_weights(inputs, nlayer=2):
    """Build the DRAM-side weight dict fed to the device program."""
    d = {}
    d["pos"] = np.ascontiguousarray(
        np.asarray(inputs["pos_embed"], np.float32)[0].T)  # [C, N]
    wq, wk, wv, wp, bq, bk, bp2 = [], [], [], [], [], [], []
    lng, lnb = [], []
    for l in range(nlayer):
        qkvw = np.asarray(inputs["topm_qkv_w"][l], np.float32)   # [3C, C]
        qkvb = np.asarray(inputs["topm_qkv_b"][l], np.float32)   # [3C]
        pw = np.asarray(inputs["topm_proj_w"][l], np.float32)    # [C, C]
        pb = np.asarray(inputs["topm_proj_b"][l], np.float32)    # [C]
        wq.append(qkvw[0 * C:1 * C].T)   # [c_in, c_out]
        wk.append(qkvw[1 * C:2 * C].T)
        wv.append(qkvw[2 * C:3 * C].T)
        wp.append(pw.T)
        bq.append(qkvb[0 * C:1 * C, None])
        bk.append(qkvb[1 * C:2 * C, None])
        bv = qkvb[2 * C:3 * C]
        # attention rows sum to 1 => P @ (V + bv) = P@V + bv; fold into proj:
        # y = Wp @ (attnout + bv) + pb  => bias' = pb + Wp @ bv
        bp2.append((pb + pw @ bv)[:, None])
        lng.append(np.asarray(inputs["topm_norm_g"][l], np.float32)[:, None])
        lnb.append(np.asarray(inputs["topm_norm_b"][l], np.float32)[:, None])
    d["wq"] = np.stack(wq).astype(BF)
    d["wk"] = np.stack(wk).astype(BF)
    d["wv"] = np.stack(wv).astype(BF)
    d["wp"] = np.stack(wp).astype(BF)
    d["bq"] = np.stack(bq).astype(np.float32)
    d["bk"] = np.stack(bk).astype(np.float32)
    d["bp2"] = np.stack(bp2).astype(np.float32)
    d["lng"] = np.stack(lng).astype(np.float32)
    d["lnb"] = np.stack(lnb).astype(np.float32)
    d["ident"] = np.eye(128, dtype=np.float32).astype(BF)
    d["identf"] = np.eye(128, dtype=np.float32)
    return d


def device_stage_ref(feat, d, nlayer=2, trace=None):
    """Numpy mirror of the device program. feat: [nseq, C, N] fp32.
    Returns pooled [C, nseq] fp32."""
    nseq = feat.shape[0]
    pooled = np.zeros((C, nseq), np.float32)
    SCL = float(HD) ** -0.5
    for s in range(nseq):
        x = bf16(feat[s] + d["pos"])                     # [C, N] fm
        for l in range(nlayer):
            wqf = d["wq"][l].astype(np.float32)
            wkf = d["wk"][l].astype(np.float32)
            wvf = d["wv"][l].astype(np.float32)
            wpf = d["wp"][l].astype(np.float32)
            xb = bf16(x)
            q = bf16((wqf.T @ xb + d["bq"][l]) * SCL)    # [o, t]
            k = bf16(wkf.T @ xb + d["bk"][l])
            v = bf16((xb.T @ wvf))                       # [t, o] token-major
            attn = np.zeros((C, N), np.float32)
            zrow = np.zeros((H, N), np.float32)
            trow = np.zeros((H, N), np.float32)
            trow_lo = np.zeros((H, N), np.float32)
            for h in range(H):
                qh = q[h * HD:(h + 1) * HD]              # [32, 512]
                kh = k[h * HD:(h + 1) * HD]
                s_tm = qh.T @ kh                         # [q, k] fp32
                segs = s_tm.reshape(N, 8, 64)
                cand = -np.sort(-segs, axis=-1)[:, :, :8].reshape(N, 64)
                top24 = -np.sort(-cand, axis=-1)[:, :24]
                t = top24[:, 19]
                tlo = np.abs(t) * 2.0 ** -9
                tneg = tlo - t                           # = -t'
                z = np.exp(top24[:, :TOPM] + tneg[:, None]).sum(-1)
                zrow[h] = 1.0 / z
                trow[h] = bf16(tneg)
                trow_lo[h] = bf16(tlo)
                # phase 2 (k-major): s~ = s^T - t' (via bf16 rank-1 pieces)
                st = (kh.T @ qh) + trow[h][None, :] + trow_lo[h][None, :]
                e2 = bf16(np.exp(st))
                em = bf16((st >= 0.0) * e2)
                vh = v[:, h * HD:(h + 1) * HD]           # [k, 32]
                attn[h * HD:(h + 1) * HD] = vh.T @ em    # [32, q]
            zrep = np.repeat(bf16(zrow), HD, axis=0)     # [256, 512]
            attn = bf16(attn * zrep)
            y = wpf.T @ attn + d["bp2"][l]
            r = bf16(y + xb)
            mu = r.mean(axis=0, keepdims=True)
            var = r.var(axis=0, keepdims=True)
            rstd = np.exp(-0.5 * np.log(var + 1e-5))
            xh = bf16((r - mu) * rstd)
            x = d["lng"][l] * xh + d["lnb"][l]
        pooled[:, s] = x.mean(axis=1)
        if trace is not None:
            trace[s] = x
    return pooled


def final_stage(pooled_all, inputs):
    """pooled_all: [B*NC, C] fp32 -> logits [B, NC] (numpy, reference math)."""
    B, NC = 16, 3
    LN_EPS = 1e-5
    x = pooled_all.reshape(B, NC, C).astype(np.float32)

    def ln(v, g, b):
        m = v.mean(-1, keepdims=True)
        s = v.var(-1, keepdims=True)
        return (v - m) / np.sqrt(s + LN_EPS) * g + b

    for l in range(2):
        inw = np.asarray(inputs["cross_in_w"][l], np.float32)
        inb = np.asarray(inputs["cross_in_b"][l], np.float32)
        ow = np.asarray(inputs["cross_out_w"][l], np.float32)
        ob = np.asarray(inputs["cross_out_b"][l], np.float32)
        qkv = (x @ inw.T + inb).reshape(B, NC, 3, H, HD).transpose(2, 0, 3, 1, 4)
        qq, kk, vv = qkv[0], qkv[1], qkv[2]
        a = np.einsum('bhnd,bhmd->bhnm', qq, kk) * (HD ** -0.5)
        a = a - a.max(-1, keepdims=True)
        e = np.exp(a)
        p = e / e.sum(-1, keepdims=True)
        o = np.einsum('bhnm,bhmd->bhnd', p, vv).transpose(0, 2, 1, 3).reshape(B, NC, C)
        o = o @ ow.T + ob
        x = ln(x + o, np.asarray(inputs["cross_ln_g"][l], np.float32),
               np.asarray(inputs["cross_ln_b"][l], np.float32))
    hh = x @ np.asarray(inputs["cls_w1"], np.float32).T + np.asarray(inputs["cls_b1"], np.float32)
    hh = np.maximum(ln(hh, np.asarray(inputs["cls_ln1_g"], np.float32),
                       np.asarray(inputs["cls_ln1_b"], np.float32)), 0.0)
    hh = hh @ np.asarray(inputs["cls_w2"], np.float32).T + np.asarray(inputs["cls_b2"], np.float32)
    hh = np.maximum(ln(hh, np.asarray(inputs["cls_ln2_g"], np.float32),
                       np.asarray(inputs["cls_ln2_b"], np.float32)), 0.0)
    logits = (hh @ np.asarray(inputs["cls_w3"], np.float32).T
              + np.asarray(inputs["cls_b3"], np.float32))[..., 0]
    return logits


# ----------------------------------------------------------------- entry ----
NCORES = 8
_CACHE = {}


def _get_nc(nseq, nlayer, reps=1):
    key = (nseq, nlayer, reps)
    if key not in _CACHE:
        _CACHE[key] = build(nseq=nseq, nlayer=nlayer, reps=reps)
    return _CACHE[key]


def run_device_stage(feat_all, d, nseq_per_core=6, nlayer=2, reps=1):
    nc = _get_nc(nseq_per_core, nlayer, reps)
    base = {k: np.ascontiguousarray(d[k]) for k in
            ("pos", "wq", "wk", "wv", "wp", "bq", "bk", "bp2", "lng", "lnb",
             "ident", "identf")}
    in_maps = []
    for k in range(NCORES):
        m = dict(base)
        m["feat"] = np.ascontiguousarray(
            feat_all[k * nseq_per_core:(k + 1) * nseq_per_core])
        in_maps.append(m)
    res = run_bass_kernel_spmd(nc, in_maps, core_ids=list(range(NCORES)))
    pooled = np.concatenate(
        [res.results[k]["pooled"].T for k in range(NCORES)], axis=0)
    return pooled, res


def kernel(**inputs):
    inputs = {k: np.asarray(v) for k, v in inputs.items()}
    d = prep_weights(inputs)
    feat_all = np.asarray(inputs["reweighted_features"], np.float32)
    pooled, _ = run_device_stage(feat_all, d)
    logits = final_stage(pooled, inputs)
    return logits.astype(np.float32)



# revision 7
# speedup vs baseline: 3.7173x; 3.7173x over previous
"""Trainium2 Bass kernel for nn_EnhancedClassificationHead_53944789237866.

kernel(**inputs) takes the full unsharded inputs (as from setup_inputs())
and returns the full (16, 3) logits.  The heavy stage (2 top-20-masked
attention layers over 48 sequences of 512 tokens) runs on 8 NeuronCores,
batch-sharded 6 sequences per core; the tiny cross-attention + classifier
tail (48 tokens total) runs on host.
"""

import sys, os, json
for _p in ("/opt/trn_rl_repo",):
    if _p not in sys.path:
        sys.path.insert(0, _p)
import numpy as np
import ml_dtypes
from contextlib import ExitStack

# ---------------------------------------------------------------- compat ----


def fix_bir_multiwait(bir_bytes):
    m = json.loads(bir_bytes)
    changed = False
    for f in m.get("functions", []):
        for bb in f.get("basicblocks", f.get("blocks", [])):
            insts = bb["instructions"]
            out = []
            for inst in insts:
                si = inst.get("sync_info")
                waits = (si or {}).get("on_wait") or []
                if len(waits) > 1:
                    changed = True
                    for k, w in enumerate(waits[:-1]):
                        out.append({
                            "debug": inst.get("debug"),
                            "engine": inst["engine"],
                            "ins": [], "outs": [],
                            "name": f"{inst['name']}-ws{k}",
                            "opcode": "Drain",
                            "sync_info": {"on_update": [], "on_wait": [w]},
                        })
                    si["on_wait"] = [waits[-1]]
                out.append(inst)
            bb["instructions"] = out
    if not changed:
        return bir_bytes
    return json.dumps(m).encode()


_installed = False


def install():
    global _installed
    if _installed:
        return
    _installed = True
    import concourse.bass_utils as bu
    import concourse.bass2jax as b2j

    orig = bu.compile_bir_kernel

    def wrapped(bir_json, tmpdir, neff_name="file.neff"):
        return orig(fix_bir_multiwait(bir_json), tmpdir, neff_name)

    bu.compile_bir_kernel = wrapped
    b2j.compile_bir_kernel = wrapped


install()

import concourse.bass as bass
import concourse.mybir as mybir
import concourse.tile as tile
from concourse.bass_utils import run_bass_kernel_spmd

# --------------------------------------------------------------- builder ----


F32 = mybir.dt.float32
BF16 = mybir.dt.bfloat16
FP16 = mybir.dt.float16
AX = mybir.AxisListType
ALU = mybir.AluOpType
ACTF = mybir.ActivationFunctionType

C = 256
N = 512
H = 8
HD = 32
TOPM = 20
NCT = 2   # c tiles (256/128)
NQT = 4   # q chunks (512/128)
P = 128


def build(nseq=6, nlayer=2, reps=1):
    nc = bass.Bass()

    feat = nc.dram_tensor("feat", [nseq, C, N], F32, kind="ExternalInput")
    pos = nc.dram_tensor("pos", [C, N], F32, kind="ExternalInput")
    wq = nc.dram_tensor("wq", [nlayer, C, C], BF16, kind="ExternalInput")
    wk = nc.dram_tensor("wk", [nlayer, C, C], BF16, kind="ExternalInput")
    wv = nc.dram_tensor("wv", [nlayer, C, C], BF16, kind="ExternalInput")
    wp = nc.dram_tensor("wp", [nlayer, C, C], BF16, kind="ExternalInput")
    bq = nc.dram_tensor("bq", [nlayer, C, 1], F32, kind="ExternalInput")
    bk = nc.dram_tensor("bk", [nlayer, C, 1], F32, kind="ExternalInput")
    bp2 = nc.dram_tensor("bp2", [nlayer, C, 1], F32, kind="ExternalInput")
    lng = nc.dram_tensor("lng", [nlayer, C, 1], F32, kind="ExternalInput")
    lnb = nc.dram_tensor("lnb", [nlayer, C, 1], F32, kind="ExternalInput")
    ident = nc.dram_tensor("ident", [128, 128], BF16, kind="ExternalInput")
    identf = nc.dram_tensor("identf", [128, 128], F32, kind="ExternalInput")
    z10d = nc.dram_tensor("z10d", [128, 128], BF16, kind="ExternalInput")
    z01d = nc.dram_tensor("z01d", [128, 128], BF16, kind="ExternalInput")
    pooled = nc.dram_tensor("pooled", [C, nseq], F32, kind="ExternalOutput")

    with ExitStack() as ctx:
        tc = ctx.enter_context(tile.TileContext(nc))
        _body(ctx, tc, nseq, nlayer, reps, feat, pos, wq, wk, wv, wp,
              bq, bk, bp2, lng, lnb, ident, identf, z10d, z01d, pooled)
    return nc


def _body(ctx, tc, nseq, nlayer, reps, feat, pos, wq, wk, wv, wp,
          bq, bk, bp2, lng, lnb, ident, identf, z10d, z01d, pooled):
    nc = tc.nc
    SCL = float(HD) ** -0.5

    consts = ctx.enter_context(tc.tile_pool(name="consts", bufs=1))
    xpool = ctx.enter_context(tc.tile_pool(name="xpool", bufs=1))
    qkv = ctx.enter_context(tc.tile_pool(name="qkv", bufs=1))
    work = ctx.enter_context(tc.tile_pool(name="work", bufs=1))
    small = ctx.enter_context(tc.tile_pool(name="small", bufs=1))
    psBig = ctx.enter_context(tc.tile_pool(name="psBig", bufs=1, space="PSUM"))
    psSm = ctx.enter_context(tc.tile_pool(name="psSm", bufs=1, space="PSUM"))

    # ---- constants ----
    id_f = consts.tile([128, 128], F32)
    nc.sync.dma_start(out=id_f, in_=identf[:, :])
    ones_bf = consts.tile([128, 128], BF16)
    nc.vector.memset(ones_bf, 1.0)
    onecol_bf = consts.tile([128, 1], BF16)
    nc.vector.memset(onecol_bf, 1.0)
    # selector patterns: rows 32j vs rows 32j+1 (host-built)
    z10 = consts.tile([128, 128], BF16)
    nc.sync.dma_start(out=z10, in_=z10d[:, :])
    z01 = consts.tile([128, 128], BF16)
    nc.sync.dma_start(out=z01, in_=z01d[:, :])
    zero_c = consts.tile([P, 1], F32)
    nc.vector.memset(zero_c, 0.0)
    eps_c = consts.tile([P, 1], F32)
    nc.vector.memset(eps_c, 1e-5)
    pos_t = [consts.tile([P, N], F32, tag=f"pos{_}", name=f"pos{_}") for _ in range(NCT)]
    for c in range(NCT):
        nc.sync.dma_start(out=pos_t[c], in_=pos[c * P:(c + 1) * P, :])

    W = {}
    for name, src in (("wq", wq), ("wk", wk), ("wv", wv), ("wp", wp)):
        W[name] = {}
        for l in range(nlayer):
            W[name][l] = [consts.tile([P, C], BF16, tag=f"{name}_{l}_{_}", name=f"{name}_{l}_{_}") for _ in range(NCT)]
            for c in range(NCT):
                nc.sync.dma_start(out=W[name][l][c],
                                  in_=src[l, c * P:(c + 1) * P, :])
    B = {}
    for name, src in (("bq", bq), ("bk", bk), ("bp2", bp2),
                      ("lng", lng), ("lnb", lnb)):
        B[name] = {}
        for l in range(nlayer):
            B[name][l] = [consts.tile([P, 1], F32, tag=f"{name}_{l}_{_}", name=f"{name}_{l}_{_}") for _ in range(NCT)]
            for c in range(NCT):
                nc.sync.dma_start(out=B[name][l][c],
                                  in_=src[l, c * P:(c + 1) * P, :])

    x_fm = [xpool.tile([P, nseq, N], BF16, tag=f"x_fm{_}", name=f"x_fm{_}")
            for _ in range(NCT)]
    pooled_sb = xpool.tile([P, NCT, nseq], F32, tag="pooled")

    def bcast_mid(ap, n):
        # insert a stride-0 dim after the partition dim
        a = list(ap.ap)
        return bass.AP(tensor=ap.tensor, offset=ap.offset,
                       ap=[a[0]] + [[0, n]] + a[1:])

    for _rep in range(reps):
        for c in range(NCT):
            x_ld = xpool.tile([P, nseq, N], F32, tag="x_ld", name=f"x_ld{c}")
            for s in range(nseq):
                nc.sync.dma_start(out=x_ld[:, s, :],
                                  in_=feat[s, c * P:(c + 1) * P, :])
            nc.vector.tensor_add(x_fm[c], x_ld, bcast_mid(pos_t[c][:, :], nseq))
        for l in range(nlayer):
            _layer(tc, l, nseq, x_fm, W, B, id_f, ones_bf, onecol_bf, z10, z01,
                   zero_c, eps_c, dict(qkv=qkv, work=work, small=small,
                                       psBig=psBig, psSm=psSm))
        for c in range(NCT):
            nc.vector.tensor_reduce(out=pooled_sb[:, c, :], in_=x_fm[c],
                                    axis=AX.X, op=ALU.add)

    psc = small.tile([P, NCT, nseq], F32, tag="psc")
    nc.vector.tensor_scalar_mul(psc, pooled_sb, 1.0 / N)
    for c in range(NCT):
        nc.sync.dma_start(out=pooled[c * P:(c + 1) * P, :], in_=psc[:, c, :])


def _layer(tc, l, nseq, x_fm, W, B, id_f, ones_bf, onecol_bf, z10, z01,
           zero_c, eps_c, pools):
    nc = tc.nc
    SCL = float(HD) ** -0.5
    qkv, work, small = pools["qkv"], pools["work"], pools["small"]
    psBig, psSm = pools["psBig"], pools["psSm"]
    wq_t, wk_t, wv_t, wp_t = W["wq"][l], W["wk"][l], W["wv"][l], W["wp"][l]
    bq_t, bk_t, bp2_t = B["bq"][l], B["bk"][l], B["bp2"][l]
    lng_t, lnb_t = B["lng"][l], B["lnb"][l]
    HCH = 3  # seqs per psum chunk
    NCH = (nseq + HCH - 1) // HCH

    # ---- Q, K projections (fm layout, seq-batched bias) ----
    q_fm = [qkv.tile([P, nseq, N], BF16, tag=f"q_fm{_}", name=f"q_fm{_}") for _ in range(NCT)]
    k_fm = [qkv.tile([P, nseq, N], BF16, tag=f"k_fm{_}", name=f"k_fm{_}") for _ in range(NCT)]
    for dst, wt, bt, scl in ((q_fm, wq_t, bq_t, SCL), (k_fm, wk_t, bk_t, None)):
        for o in range(NCT):
            for ch in range(NCH):
                s0, s1 = ch * HCH, min((ch + 1) * HCH, nseq)
                ps = psBig.tile([P, NQT, N], F32, tag="big")
                for s in range(s0, s1):
                    for c in range(NCT):
                        nc.tensor.matmul(ps[:, s - s0, :],
                                         lhsT=wt[c][:, o * P:(o + 1) * P],
                                         rhs=x_fm[c][:, s, :],
                                         start=(c == 0), stop=(c == NCT - 1))
                if scl is not None:
                    nc.vector.tensor_scalar(dst[o][:, s0:s1, :], ps[:, 0:s1 - s0, :],
                                            bt[o], scl, op0=ALU.add, op1=ALU.mult)
                else:
                    nc.vector.tensor_scalar(dst[o][:, s0:s1, :], ps[:, 0:s1 - s0, :],
                                            bt[o], None, op0=ALU.add)

    # ---- V (token-major) per seq ----
    v_tm = [qkv.tile([P, NQT, C], BF16, tag=f"v_tm{_}", name=f"v_tm{_}")
            for _ in range(nseq)]
    for s in range(nseq):
        ps = psBig.tile([P, NQT, N], F32, tag="big")
        for t in range(NQT):
            for c in range(NCT):
                nc.tensor.matmul(ps[:, t, 0:C],
                                 lhsT=x_fm[c][:, s, t * P:(t + 1) * P],
                                 rhs=wv_t[c], start=(c == 0), stop=(c == NCT - 1))
        nc.vector.tensor_copy(v_tm[s], ps[:, :, 0:C])

    attn_all = [qkv.tile([P, nseq, N], BF16, tag=f"attn{_}", name=f"attn{_}")
                for _ in range(NCT)]

    for s in range(nseq):
        # ---- phase 1: S (q-major) -> fp16 -> top-24 -> thr/Z cols ----
        Z24 = [small.tile([P, H, 24], FP16, tag=f"z24_{qc}", name=f"z24_{qc}")
               for qc in range(NQT)]
        ZT = small.tile([P, NQT, NCT, P], F32, tag="zt", name="zt")
        for ht in range(NCT):
            for qc in range(NQT):
                psS = psBig.tile([P, NQT, N], F32, tag="big")
                for hr in range(NQT):
                    b = hr * 32
                    nc.tensor.matmul(psS[:, hr, :],
                                     lhsT=q_fm[ht][b:b + 32, s, qc * P:(qc + 1) * P],
                                     rhs=k_fm[ht][b:b + 32, s, :],
                                     start=True, stop=True,
                                     tile_position=(b, 0))
                sfp = work.tile([P, NQT, N], FP16, tag="sfp")
                nc.vector.tensor_copy(sfp, psS)
                for hr in range(NQT):
                    g = ht * 4 + hr
                    nc.vector.max(out=Z24[qc][:, g, 0:8], in_=sfp[:, hr, :])
                    nc.vector.match_replace(out=sfp[:, hr, :],
                                            in_to_replace=Z24[qc][:, g, 0:8],
                                            in_values=sfp[:, hr, :],
                                            imm_value=-60000.0)
                    nc.vector.max(out=Z24[qc][:, g, 8:16], in_=sfp[:, hr, :])
                    nc.vector.match_replace(out=sfp[:, hr, :],
                                            in_to_replace=Z24[qc][:, g, 8:16],
                                            in_values=sfp[:, hr, :],
                                            imm_value=-60000.0)
                    nc.vector.max(out=Z24[qc][:, g, 16:24], in_=sfp[:, hr, :])
        # thr cols (-thr at col 32j) and 1/Z cols (at col 32j+1), per qc
        for qc in range(NQT):
            tv = Z24[qc][:, :, 19:20]            # [P, 8, 1] fp16
            tb = small.tile([P, H], F32, tag="tb", name=f"tb{qc}")
            nc.vector.tensor_scalar(tb, tv, -1.0, None, op0=ALU.mult)
            ta = small.tile([P, H], F32, tag="ta", name=f"ta{qc}")
            nc.vector.tensor_max(ta, tv, tb)
            # -thr = 2*tlo - t  (tlo = |t| * 2^-9)
            nc.vector.scalar_tensor_tensor(
                out=ZT[:, qc, :, 0:97:32],
                in0=ta.rearrange("p (a b) -> p a b", a=NCT), scalar=2.0 ** -8,
                in1=tv.rearrange("p (a b) t -> p a (b t)", a=NCT),
                op0=ALU.mult, op1=ALU.subtract)
            thr_ap = ZT[:, qc, :, 0:97:32]
            thr_b = bass.AP(tensor=thr_ap.tensor, offset=thr_ap.offset,
                            ap=list(thr_ap.ap) + [[0, TOPM]])
            zd = small.tile([P, NCT, NQT, TOPM], F32, tag="zd", name=f"zd{qc}")
            nc.vector.tensor_add(
                zd, Z24[qc][:, :, 0:TOPM].rearrange("p (a b) t -> p a b t", a=NCT),
                thr_b)
            zel = small.tile([P, NCT, NQT, TOPM], F32, tag="zel", name=f"zel{qc}")
            nc.scalar.activation(zel, zd, ACTF.Exp, bias=zero_c)
            nc.vector.tensor_reduce(out=ZT[:, qc, :, 1:98:32], in_=zel,
                                    axis=AX.X, op=ALU.add)
        nc.vector.reciprocal(ZT[:, :, :, 1:98:32], ZT[:, :, :, 1:98:32])

        # ---- assemble srow[g]: row 32j = -thr, row 32j+1 = 1/Z ----
        srow = []
        for g in range(NCT):
            prow = psSm.tile([P, N], F32, tag="pv0" if g == 0 else "pv1",
                             name=f"prow{g}")
            for qc in range(NQT):
                nc.tensor.transpose(prow[:, qc * P:(qc + 1) * P],
                                    ZT[:, qc, g, :], id_f)
            sr = work.tile([P, N], BF16, tag=f"srow{g}", name=f"srow{g}")
            nc.vector.tensor_copy(sr, prow)
            srow.append(sr)

        # ---- phase 2: S^T - thr -> exp/mask -> PV ----
        pv_ps = [psSm.tile([P, N], F32, tag=f"pv{i}", name=f"pvps{i}")
                 for i in range(NCT)]
        for h in range(H):
            ht, hr = divmod(h, 4)
            b = hr * 32
            psT = psBig.tile([P, NQT, N], F32, tag="big")
            for kc in range(NQT):
                nc.tensor.matmul(psT[:, kc, :],
                                 lhsT=k_fm[ht][b:b + 32, s, kc * P:(kc + 1) * P],
                                 rhs=q_fm[ht][b:b + 32, s, :],
                                 start=True, stop=False,
                                 tile_position=(b, 0))
            for kc in range(NQT):
                nc.tensor.matmul(psT[:, kc, :],
                                 lhsT=z10[b:b + 2, 0:128],
                                 rhs=srow[ht][b:b + 2, :],
                                 start=False, stop=True,
                                 tile_position=(b, 0))
            et = work.tile([P, NQT, N], BF16, tag="et")
            nc.scalar.activation(et, psT, ACTF.Exp, bias=zero_c)
            em = work.tile([P, NQT, N], BF16, tag="em")
            nc.vector.scalar_tensor_tensor(out=em, in0=psT, scalar=0.0,
                                           in1=et, op0=ALU.is_ge, op1=ALU.mult)
            for kc in range(NQT):
                nc.tensor.matmul(pv_ps[ht][b:b + 32, :],
                                 lhsT=v_tm[s][:, kc, h * 32:(h + 1) * 32],
                                 rhs=em[:, kc, :],
                                 start=(kc == 0), stop=(kc == NQT - 1),
                                 tile_position=(0, b))
        # zrep: broadcast 1/Z row to 32 rows per head; attn = pv * zrep
        zrep_ps = [psSm.tile([P, N], F32, tag=f"zr{i}", name=f"zrps{i}")
                   for i in range(NCT)]
        for h in range(H):
            ht, hr = divmod(h, 4)
            b = hr * 32
            nc.tensor.matmul(zrep_ps[ht][b:b + 32, :],
                             lhsT=z01[b:b + 2, 0:32],
                             rhs=srow[ht][b:b + 2, :],
                             start=True, stop=True, tile_position=(b, b))
        for c in range(NCT):
            zr_sb = work.tile([P, N], BF16, tag="zr_sb")
            nc.vector.tensor_copy(zr_sb, zrep_ps[c])
            nc.vector.tensor_mul(attn_all[c][:, s, :], pv_ps[c], zr_sb)

    # ---- proj + residual (seq-batched) ----
    r_fm = [work.tile([P, nseq, N], BF16, tag=f"r_fm{_}", name=f"r_fm{_}")
            for _ in range(NCT)]
    for o in range(NCT):
        for ch in range(NCH):
            s0, s1 = ch * HCH, min((ch + 1) * HCH, nseq)
            ps = psBig.tile([P, NQT, N], F32, tag="big")
            for s in range(s0, s1):
                for c in range(NCT):
                    nc.tensor.matmul(ps[:, s - s0, :],
                                     lhsT=wp_t[c][:, o * P:(o + 1) * P],
                                     rhs=attn_all[c][:, s, :],
                                     start=(c == 0), stop=(c == NCT - 1))
            nc.vector.scalar_tensor_tensor(
                out=r_fm[o][:, s0:s1, :], in0=ps[:, 0:s1 - s0, :],
                scalar=bp2_t[o], in1=x_fm[o][:, s0:s1, :],
                op0=ALU.add, op1=ALU.add)

    # ---- LayerNorm over channels (matmul stats, seq-batched) ----
    stat = [small.tile([1, nseq, N], F32, tag=f"stat{_}", name=f"stat{_}")
            for _ in range(2)]
    rsq = [work.tile([P, nseq, N], BF16, tag=f"rsq{_}", name=f"rsq{_}")
           for _ in range(NCT)]
    for c in range(NCT):
        nc.vector.tensor_mul(rsq[c], r_fm[c], r_fm[c])
    for k, srcs in ((0, r_fm), (1, rsq)):
        for ch in range(NCH):
            s0, s1 = ch * HCH, min((ch + 1) * HCH, nseq)
            ps = psBig.tile([P, NQT, N], F32, tag="big")
            for s in range(s0, s1):
                for c in range(NCT):
                    nc.tensor.matmul(ps[0:1, s - s0, :], lhsT=onecol_bf,
                                     rhs=srcs[c][:, s, :],
                                     start=(c == 0), stop=(c == NCT - 1))
            nc.vector.tensor_scalar_mul(stat[k][:, s0:s1, :],
                                        ps[0:1, 0:s1 - s0, :], 1.0 / C)
    rowA = small.tile([1, nseq, N], F32, tag="rowA", name="rowA")
    rowB = small.tile([1, nseq, N], F32, tag="rowB", name="rowB")
    nc.vector.tensor_mul(rowA, stat[0], stat[0])
    nc.vector.tensor_sub(rowB, stat[1], rowA)
    nc.scalar.activation(rowA, rowB, ACTF.Ln, bias=eps_c[0:1, :])
    nc.scalar.activation(rowB, rowA, ACTF.Exp, bias=zero_c[0:1, :], scale=-0.5)
    mrow = small.tile([1, nseq, N], BF16, tag="mrow", name="mrow")
    nc.vector.tensor_copy(mrow, stat[0])
    rrow = small.tile([1, nseq, N], BF16, tag="rrow", name="rrow")
    nc.vector.tensor_copy(rrow, rowB)
    t1 = [work.tile([P, nseq, N], BF16, tag=f"t1_{_}", name=f"t1_{_}")
          for _ in range(NCT)]
    for ch in range(NCH):
        s0, s1 = ch * HCH, min((ch + 1) * HCH, nseq)
        ps = psBig.tile([P, NQT, N], F32, tag="big")
        for s in range(s0, s1):
            nc.tensor.matmul(ps[:, s - s0, :], lhsT=ones_bf[0:1, :],
                             rhs=mrow[0:1, s, :], start=True, stop=True)
        for c in range(NCT):
            nc.vector.tensor_sub(t1[c][:, s0:s1, :], r_fm[c][:, s0:s1, :],
                                 ps[:, 0:s1 - s0, :])
    for ch in range(NCH):
        s0, s1 = ch * HCH, min((ch + 1) * HCH, nseq)
        ps = psBig.tile([P, NQT, N], F32, tag="big")
        for s in range(s0, s1):
            nc.tensor.matmul(ps[:, s - s0, :], lhsT=ones_bf[0:1, :],
                             rhs=rrow[0:1, s, :], start=True, stop=True)
        for c in range(NCT):
            xh = work.tile([P, HCH, N], BF16, tag="xh")
            nc.vector.tensor_mul(xh[:, 0:s1 - s0, :], t1[c][:, s0:s1, :],
                                 ps[:, 0:s1 - s0, :])
            nc.vector.tensor_scalar(x_fm[c][:, s0:s1, :], xh[:, 0:s1 - s0, :],
                                    lng_t[c], lnb_t[c],
                                    op0=ALU.mult, op1=ALU.add)


# ----------------------------------------------------------- host helpers ----

BF = ml_dtypes.bfloat16


def bf16(x):
    return np.asarray(x, dtype=np.float32).astype(BF).astype(np.float32)


def prep_weights(inputs, nlayer=2):
    """Build the DRAM-side weight dict fed to the device program."""
    d = {}
    d["pos"] = np.ascontiguousarray(
        np.asarray(inputs["pos_embed"], np.float32)[0].T)  # [C, N]
    wq, wk, wv, wp, bq, bk, bp2 = [], [], [], [], [], [], []
    lng, lnb = [], []
    for l in range(nlayer):
        qkvw = np.asarray(inputs["topm_qkv_w"][l], np.float32)   # [3C, C]
        qkvb = np.asarray(inputs["topm_qkv_b"][l], np.float32)   # [3C]
        pw = np.asarray(inputs["topm_proj_w"][l], np.float32)    # [C, C]
        pb = np.asarray(inputs["topm_proj_b"][l], np.float32)    # [C]
        wq.append(qkvw[0 * C:1 * C].T)   # [c_in, c_out]
        wk.append(qkvw[1 * C:2 * C].T)
        wv.append(qkvw[2 * C:3 * C].T)
        wp.append(pw.T)
        bq.append(qkvb[0 * C:1 * C, None])
        bk.append(qkvb[1 * C:2 * C, None])
        bv = qkvb[2 * C:3 * C]
        # attention rows sum to 1 => P @ (V + bv) = P@V + bv; fold into proj:
        # y = Wp @ (attnout + bv) + pb  => bias' = pb + Wp @ bv
        bp2.append((pb + pw @ bv)[:, None])
        lng.append(np.asarray(inputs["topm_norm_g"][l], np.float32)[:, None])
        lnb.append(np.asarray(inputs["topm_norm_b"][l], np.float32)[:, None])
    d["wq"] = np.stack(wq).astype(BF)
    d["wk"] = np.stack(wk).astype(BF)
    d["wv"] = np.stack(wv).astype(BF)
    d["wp"] = np.stack(wp).astype(BF)
    d["bq"] = np.stack(bq).astype(np.float32)
    d["bk"] = np.stack(bk).astype(np.float32)
    d["bp2"] = np.stack(bp2).astype(np.float32)
    d["lng"] = np.stack(lng).astype(np.float32)
    d["lnb"] = np.stack(lnb).astype(np.float32)
    d["ident"] = np.eye(128, dtype=np.float32).astype(BF)
    d["identf"] = np.eye(128, dtype=np.float32)
    z10 = np.ones((128, 128), np.float32)
    z10[1::32, :] = 0.0
    z01 = np.zeros((128, 128), np.float32)
    z01[1::32, :] = 1.0
    d["z10d"] = z10.astype(BF)
    d["z01d"] = z01.astype(BF)
    return d


def device_stage_ref(feat, d, nlayer=2, trace=None):
    """Numpy mirror of the device program. feat: [nseq, C, N] fp32.
    Returns pooled [C, nseq] fp32."""
    nseq = feat.shape[0]
    pooled = np.zeros((C, nseq), np.float32)
    SCL = float(HD) ** -0.5
    for s in range(nseq):
        x = bf16(feat[s] + d["pos"])                     # [C, N] fm
        for l in range(nlayer):
            wqf = d["wq"][l].astype(np.float32)
            wkf = d["wk"][l].astype(np.float32)
            wvf = d["wv"][l].astype(np.float32)
            wpf = d["wp"][l].astype(np.float32)
            xb = bf16(x)
            q = bf16((wqf.T @ xb + d["bq"][l]) * SCL)    # [o, t]
            k = bf16(wkf.T @ xb + d["bk"][l])
            v = bf16((xb.T @ wvf))                       # [t, o] token-major
            attn = np.zeros((C, N), np.float32)
            zrow = np.zeros((H, N), np.float32)
            trow = np.zeros((H, N), np.float32)
            trow_lo = np.zeros((H, N), np.float32)
            for h in range(H):
                qh = q[h * HD:(h + 1) * HD]              # [32, 512]
                kh = k[h * HD:(h + 1) * HD]
                s_tm = qh.T @ kh                         # [q, k] fp32
                segs = s_tm.reshape(N, 8, 64)
                cand = -np.sort(-segs, axis=-1)[:, :, :8].reshape(N, 64)
                top24 = -np.sort(-cand, axis=-1)[:, :24]
                t = top24[:, 19]
                tlo = np.abs(t) * 2.0 ** -9
                tneg = tlo - t                           # = -t'
                z = np.exp(top24[:, :TOPM] + tneg[:, None]).sum(-1)
                zrow[h] = 1.0 / z
                trow[h] = bf16(tneg)
                trow_lo[h] = bf16(tlo)
                # phase 2 (k-major): s~ = s^T - t' (via bf16 rank-1 pieces)
                st = (kh.T @ qh) + trow[h][None, :] + trow_lo[h][None, :]
                e2 = bf16(np.exp(st))
                em = bf16((st >= 0.0) * e2)
                vh = v[:, h * HD:(h + 1) * HD]           # [k, 32]
                attn[h * HD:(h + 1) * HD] = vh.T @ em    # [32, q]
            zrep = np.repeat(bf16(zrow), HD, axis=0)     # [256, 512]
            attn = bf16(attn * zrep)
            y = wpf.T @ attn + d["bp2"][l]
            r = bf16(y + xb)
            mu = r.mean(axis=0, keepdims=True)
            var = r.var(axis=0, keepdims=True)
            rstd = np.exp(-0.5 * np.log(var + 1e-5))
            xh = bf16((r - mu) * rstd)
            x = d["lng"][l] * xh + d["lnb"][l]
        pooled[:, s] = x.mean(axis=1)
        if trace is not None:
            trace[s] = x
    return pooled


def final_stage(pooled_all, inputs):
    """pooled_all: [B*NC, C] fp32 -> logits [B, NC] (numpy, reference math)."""
    B, NC = 16, 3
    LN_EPS = 1e-5
    x = pooled_all.reshape(B, NC, C).astype(np.float32)

    def ln(v, g, b):
        m = v.mean(-1, keepdims=True)
        s = v.var(-1, keepdims=True)
        return (v - m) / np.sqrt(s + LN_EPS) * g + b

    for l in range(2):
        inw = np.asarray(inputs["cross_in_w"][l], np.float32)
        inb = np.asarray(inputs["cross_in_b"][l], np.float32)
        ow = np.asarray(inputs["cross_out_w"][l], np.float32)
        ob = np.asarray(inputs["cross_out_b"][l], np.float32)
        qkv = (x @ inw.T + inb).reshape(B, NC, 3, H, HD).transpose(2, 0, 3, 1, 4)
        qq, kk, vv = qkv[0], qkv[1], qkv[2]
        a = np.einsum('bhnd,bhmd->bhnm', qq, kk) * (HD ** -0.5)
        a = a - a.max(-1, keepdims=True)
        e = np.exp(a)
        p = e / e.sum(-1, keepdims=True)
        o = np.einsum('bhnm,bhmd->bhnd', p, vv).transpose(0, 2, 1, 3).reshape(B, NC, C)
        o = o @ ow.T + ob
        x = ln(x + o, np.asarray(inputs["cross_ln_g"][l], np.float32),
               np.asarray(inputs["cross_ln_b"][l], np.float32))
    hh = x @ np.asarray(inputs["cls_w1"], np.float32).T + np.asarray(inputs["cls_b1"], np.float32)
    hh = np.maximum(ln(hh, np.asarray(inputs["cls_ln1_g"], np.float32),
                       np.asarray(inputs["cls_ln1_b"], np.float32)), 0.0)
    hh = hh @ np.asarray(inputs["cls_w2"], np.float32).T + np.asarray(inputs["cls_b2"], np.float32)
    hh = np.maximum(ln(hh, np.asarray(inputs["cls_ln2_g"], np.float32),
                       np.asarray(inputs["cls_ln2_b"], np.float32)), 0.0)
    logits = (hh @ np.asarray(inputs["cls_w3"], np.float32).T
              + np.asarray(inputs["cls_b3"], np.float32))[..., 0]
    return logits


# ----------------------------------------------------------------- entry ----
NCORES = 8
_CACHE = {}


def _get_nc(nseq, nlayer, reps=1):
    key = (nseq, nlayer, reps)
    if key not in _CACHE:
        _CACHE[key] = build(nseq=nseq, nlayer=nlayer, reps=reps)
    return _CACHE[key]


def run_device_stage(feat_all, d, nseq_per_core=6, nlayer=2, reps=1):
    nc = _get_nc(nseq_per_core, nlayer, reps)
    base = {k: np.ascontiguousarray(d[k]) for k in
            ("pos", "wq", "wk", "wv", "wp", "bq", "bk", "bp2", "lng", "lnb",
             "ident", "identf", "z10d", "z01d")}
    in_maps = []
    for k in range(NCORES):
        m = dict(base)
        m["feat"] = np.ascontiguousarray(
            feat_all[k * nseq_per_core:(k + 1) * nseq_per_core])
        in_maps.append(m)
    res = run_bass_kernel_spmd(nc, in_maps, core_ids=list(range(NCORES)))
    pooled = np.concatenate(
        [res.results[k]["pooled"].T for k in range(NCORES)], axis=0)
    return pooled, res


def kernel(**inputs):
    inputs = {k: np.asarray(v) for k, v in inputs.items()}
    d = prep_weights(inputs)
    feat_all = np.asarray(inputs["reweighted_features"], np.float32)
    pooled, _ = run_device_stage(feat_all, d)
    logits = final_stage(pooled, inputs)
    return logits.astype(np.float32)

